# revision 8
# baseline (speedup 1.0000x reference)
"""Bilateral filter (7x7, sigma_color=0.1) Trainium2 Bass kernel.

Full inputs: input_tensor [16, 3, 1024, 1024] fp32 in [0,1].
Sharding: batch-parallel — 48 channel-images split as 6 per core across 8 cores.

Per-core algorithm (one For_i loop over the 6 channel-images):
  1. Build an edge-padded copy [H+6, W+6] in a DRAM-pool tile via DMAs.
  2. For each 128-row band, DMA 7 row-shifted tiles T_dy [128, W+6] from the
     padded image (compute engines cannot shift partitions, DMA can).
  3. Per tap (dy,dx) != center:  d = T_dy[:, dx:dx+W] - center;  q = d*d;
     w = exp(-50*q - ds2/18)  (ACT, spatial weight folded into bias);
     num += w*s (DVE);  den += w (GpSimd).
  4. out = num * approx_reciprocal(den); DMA band to output.
"""

import sys

sys.path.insert(0, "/opt/trn_rl_repo")

import numpy as np

SPATIAL_RADIUS = 3
COLOR_RADIUS = 0.1
INV_2C2 = 1.0 / (2.0 * COLOR_RADIUS**2)  # 50.0
INV_2R2 = 1.0 / (2.0 * float(SPATIAL_RADIUS) ** 2)  # 1/18

N_CORES = 8
_NC_CACHE = {}


def build_nc(n_img, H, W, variant="bf16A"):
    """Build the per-core Bass kernel: n_img channel-images of [H, W].

    variant:
      "fp32"  — all fp32: num/den accumulation of w and w*s.
      "bf16A" — A-formulation out = cen + (sum w*d)/(1 + sum w); d/q/w/t in
                bf16 (DVE 2x mode on the squares/products), A/den in fp32.
    """
    import concourse.bacc as bacc
    import concourse.bass as bass
    import concourse.mybir as mybir
    from concourse.tile import TileContext

    ds = bass.ds
    f32 = mybir.dt.float32
    bf16 = mybir.dt.bfloat16
    K = 2 * SPATIAL_RADIUS + 1  # 7
    R = SPATIAL_RADIUS  # 3
    Wp = W + 2 * R  # padded width
    P = 128  # band height (partitions)
    assert H % P == 0
    n_bands = H // P

    nc = bacc.Bacc(None, target_bir_lowering=False)
    x = nc.declare_dram_parameter("x", [n_img * H, W], f32, isOutput=False)
    y = nc.declare_dram_parameter("y", [n_img * H, W], f32, isOutput=True)

    # distinct spatial-weight classes: ds2 = (dy-R)^2 + (dx-R)^2
    ds2_vals = sorted({(dy - R) ** 2 + (dx - R) ** 2 for dy in range(K) for dx in range(K)} - {0})
    ds2_col = {v: i for i, v in enumerate(ds2_vals)}

    with TileContext(nc) as tc:
        with (
            tc.tile_pool(name="consts", bufs=1) as cpool,
            tc.tile_pool(name="drampool", bufs=2, space="DRAM") as dpool,
            tc.tile_pool(name="bandpool", bufs=2) as bpool,
            tc.tile_pool(name="workpool", bufs=2) as wpool,
            tc.tile_pool(name="accpool", bufs=2) as apool,
        ):
            bias = cpool.tile([P, len(ds2_vals)], f32)
            for v, i in ds2_col.items():
                nc.gpsimd.memset(bias[:, i : i + 1], -float(v) * INV_2R2)

            with tc.For_i(0, n_img * H, H) as gbase:
                pad = dpool.tile([H + 2 * R, Wp], f32, tag="pad")
                # ---- phase 0: build padded image in DRAM ----
                nc.sync.dma_start(out=pad[R : H + R, R : W + R], in_=x[ds(gbase, H), :])
                with nc.allow_non_contiguous_dma(reason="tiny edge-column pads"):
                    for ccol in range(R):
                        nc.sync.dma_start(out=pad[R : H + R, ccol : ccol + 1], in_=x[ds(gbase, H), 0:1])
                        nc.sync.dma_start(
                            out=pad[R : H + R, W + R + ccol : W + R + ccol + 1],
                            in_=x[ds(gbase, H), W - 1 : W],
                        )
                for rrow in range(R):
                    nc.sync.dma_start(out=pad[rrow : rrow + 1, :], in_=pad[R : R + 1, :])
                    nc.sync.dma_start(
                        out=pad[H + R + rrow : H + R + rrow + 1, :],
                        in_=pad[H + R - 1 : H + R, :],
                    )

                # ---- phase 1: bands ----
                for b in range(n_bands):
                    r0 = b * P
                    T = []
                    for dy in range(K):
                        t_dy = bpool.tile([P, Wp], f32, tag=f"T{dy}")
                        nc.sync.dma_start(out=t_dy[:, :], in_=pad[r0 + dy : r0 + dy + P, :])
                        T.append(t_dy)
                    cen = T[R][:, R : R + W]

                    if variant == "fp32":
                        num = apool.tile([P, W], f32, tag="num")
                        den = apool.tile([P, W], f32, tag="den")
                        nc.scalar.copy(num[:, :], cen)
                        nc.gpsimd.memset(den[:, :], 1.0)

                        for dy in range(K):
                            for dx in range(K):
                                if dy == R and dx == R:
                                    continue
                                s = T[dy][:, dx : dx + W]
                                q = wpool.tile([P, W], f32, tag="q")
                                w = wpool.tile([P, W], f32, tag="w")
                                t = wpool.tile([P, W], f32, tag="t")
                                nc.vector.tensor_tensor(out=q[:, :], in0=s, in1=cen, op=mybir.AluOpType.subtract)
                                nc.vector.tensor_tensor(out=q[:, :], in0=q[:, :], in1=q[:, :], op=mybir.AluOpType.mult)
                                ds2 = (dy - R) ** 2 + (dx - R) ** 2
                                nc.scalar.activation(
                                    w[:, :],
                                    q[:, :],
                                    mybir.ActivationFunctionType.Exp,
                                    bias=bias[:, ds2_col[ds2] : ds2_col[ds2] + 1],
                                    scale=-INV_2C2,
                                )
                                nc.vector.tensor_tensor(out=t[:, :], in0=w[:, :], in1=s, op=mybir.AluOpType.mult)
                                nc.vector.tensor_tensor(out=num[:, :], in0=num[:, :], in1=t[:, :], op=mybir.AluOpType.add)
                                nc.gpsimd.tensor_tensor(out=den[:, :], in0=den[:, :], in1=w[:, :], op=mybir.AluOpType.add)

                        rcp = wpool.tile([P, W], f32, tag="rcp")
                        scr = wpool.tile([P, W], f32, tag="scr")
                        nc.vector.reciprocal_approx_accurate(rcp[:, :], den[:, :], scr[:, :])
                        nc.vector.tensor_tensor(out=num[:, :], in0=num[:, :], in1=rcp[:, :], op=mybir.AluOpType.mult)
                        nc.sync.dma_start(out=y[ds(gbase + r0, P), :], in_=num[:, :])
                    else:  # bf16A
                        acc = apool.tile([P, W], f32, tag="acc")
                        den = apool.tile([P, W], f32, tag="den")
                        nc.vector.memset(acc[:, :], 0.0)
                        nc.gpsimd.memset(den[:, :], 1.0)

                        taps = [(dy, dx) for dy in range(K) for dx in range(K) if not (dy == R and dx == R)]
                        GRP = 8  # taps per bf16 partial-sum tree
                        sub_flip = 0

                        def fold_push(stack, tile, eng):
                            # binary-counter balanced fold: stack holds (level, tile)
                            lv = 0
                            while stack and stack[-1][0] == lv:
                                _, prev = stack.pop()
                                eng.tensor_tensor(out=prev[:, :], in0=prev[:, :], in1=tile[:, :], op=mybir.AluOpType.add)
                                tile = prev
                                lv += 1
                            stack.append((lv, tile))

                        for g0 in range(0, len(taps), GRP):
                            group = taps[g0 : g0 + GRP]
                            tstack, wstack = [], []
                            for gi, (dy, dx) in enumerate(group):
                                s = T[dy][:, dx : dx + W]
                                d = wpool.tile([P, W], bf16, tag=f"d{gi % 2}")
                                q = wpool.tile([P, W], bf16, tag="q")
                                w = wpool.tile([P, W], bf16, tag=f"w{gi % 4}")
                                t = wpool.tile([P, W], bf16, tag=f"t{gi % 4}")
                                # d = s - cen  (fp32 in, bf16 out); 1/3 of subs on gpsimd
                                sub_eng = nc.gpsimd if (sub_flip % 3 == 2) else nc.vector
                                sub_flip += 1
                                sub_eng.tensor_tensor(out=d[:, :], in0=s, in1=cen, op=mybir.AluOpType.subtract)
                                nc.vector.tensor_tensor(out=q[:, :], in0=d[:, :], in1=d[:, :], op=mybir.AluOpType.mult)
                                ds2 = (dy - R) ** 2 + (dx - R) ** 2
                                nc.scalar.activation(
                                    w[:, :],
                                    q[:, :],
                                    mybir.ActivationFunctionType.Exp,
                                    bias=bias[:, ds2_col[ds2] : ds2_col[ds2] + 1],
                                    scale=-INV_2C2,
                                )
                                nc.vector.tensor_tensor(out=t[:, :], in0=w[:, :], in1=d[:, :], op=mybir.AluOpType.mult)
                                fold_push(tstack, t, nc.vector)
                                fold_push(wstack, w, nc.gpsimd)
                            # fold leftovers, then fp32 root add
                            for stack, accum, eng in ((tstack, acc, nc.vector), (wstack, den, nc.gpsimd)):
                                while len(stack) > 1:
                                    _, b2 = stack.pop()
                                    _, a2 = stack.pop()
                                    eng.tensor_tensor(out=a2[:, :], in0=a2[:, :], in1=b2[:, :], op=mybir.AluOpType.add)
                                    stack.append((99, a2))
                                eng.tensor_tensor(out=accum[:, :], in0=accum[:, :], in1=stack[0][1][:, :], op=mybir.AluOpType.add)

                        rcp = wpool.tile([P, W], f32, tag="rcp")
                        scr = wpool.tile([P, W], f32, tag="scr")
                        nc.vector.reciprocal_approx_accurate(rcp[:, :], den[:, :], scr[:, :])
                        nc.vector.tensor_tensor(out=acc[:, :], in0=acc[:, :], in1=rcp[:, :], op=mybir.AluOpType.mult)
                        nc.vector.tensor_tensor(out=acc[:, :], in0=acc[:, :], in1=cen, op=mybir.AluOpType.add)
                        nc.sync.dma_start(out=y[ds(gbase + r0, P), :], in_=acc[:, :])

    nc.finalize()
    return nc


def _get_nc(n_img, H, W, variant="bf16A"):
    key = (n_img, H, W, variant)
    if key not in _NC_CACHE:
        _NC_CACHE[key] = build_nc(n_img, H, W, variant)
    return _NC_CACHE[key]


def run_sharded(flat, n_img_per_core, H, W, trace=False, variant="bf16A"):
    """flat: [N_CORES * n_img_per_core, H, W] fp32. Returns same-shape output
    (and the BassKernelResults when trace)."""
    from concourse.bass_utils import run_bass_kernel_spmd

    nc = _get_nc(n_img_per_core, H, W, variant)
    in_maps = [
        {
            "x": np.ascontiguousarray(
                flat[c * n_img_per_core : (c + 1) * n_img_per_core].reshape(n_img_per_core * H, W)
            )
        }
        for c in range(N_CORES)
    ]
    res = run_bass_kernel_spmd(nc, in_maps, core_ids=list(range(N_CORES)), trace=trace)
    out = np.stack([res.results[c]["y"].reshape(n_img_per_core, H, W) for c in range(N_CORES)])
    return out.reshape(N_CORES * n_img_per_core, H, W), res


def kernel(input_tensor: np.ndarray) -> np.ndarray:
    input_tensor = np.asarray(input_tensor, dtype=np.float32)
    B, C, H, W = input_tensor.shape
    flat = input_tensor.reshape(B * C, H, W)
    assert (B * C) % N_CORES == 0
    out, _ = run_sharded(flat, (B * C) // N_CORES, H, W)
    return out.reshape(B, C, H, W)


# revision 11
# speedup vs baseline: 1.0268x; 1.0268x over previous
"""Bilateral filter (7x7, sigma_color=0.1) Trainium2 Bass kernel.

Full inputs: input_tensor [16, 3, 1024, 1024] fp32 in [0,1].
Sharding: batch-parallel — 48 channel-images split as 6 per core across 8 cores.

Per-core algorithm (one For_i loop over the 6 channel-images):
  1. Build an edge-padded copy [H+6, W+6] in a DRAM-pool tile via DMAs.
  2. For each 128-row band, DMA 7 row-shifted tiles T_dy [128, W+6] from the
     padded image (compute engines cannot shift partitions, DMA can).
  3. Per tap (dy,dx) != center:  d = T_dy[:, dx:dx+W] - center;  q = d*d;
     w = exp(-50*q - ds2/18)  (ACT, spatial weight folded into bias);
     num += w*s (DVE);  den += w (GpSimd).
  4. out = num * approx_reciprocal(den); DMA band to output.
"""

import sys

sys.path.insert(0, "/opt/trn_rl_repo")

import numpy as np

SPATIAL_RADIUS = 3
COLOR_RADIUS = 0.1
INV_2C2 = 1.0 / (2.0 * COLOR_RADIUS**2)  # 50.0
INV_2R2 = 1.0 / (2.0 * float(SPATIAL_RADIUS) ** 2)  # 1/18

N_CORES = 8
_NC_CACHE = {}


def build_nc(n_img, H, W, variant="bf16A"):
    """Build the per-core Bass kernel: n_img channel-images of [H, W].

    variant:
      "fp32"  — all fp32: num/den accumulation of w and w*s.
      "bf16A" — A-formulation out = cen + (sum w*d)/(1 + sum w); d/q/w/t in
                bf16 (DVE 2x mode on the squares/products), A/den in fp32.
    """
    import concourse.bacc as bacc
    import concourse.bass as bass
    import concourse.mybir as mybir
    from concourse.tile import TileContext

    ds = bass.ds
    f32 = mybir.dt.float32
    bf16 = mybir.dt.bfloat16
    K = 2 * SPATIAL_RADIUS + 1  # 7
    R = SPATIAL_RADIUS  # 3
    Wp = W + 2 * R  # padded width
    P = 128  # band height (partitions)
    assert H % P == 0
    n_bands = H // P

    nc = bacc.Bacc(None, target_bir_lowering=False)
    x = nc.declare_dram_parameter("x", [n_img * H, W], f32, isOutput=False)
    y = nc.declare_dram_parameter("y", [n_img * H, W], f32, isOutput=True)

    # distinct spatial-weight classes: ds2 = (dy-R)^2 + (dx-R)^2
    ds2_vals = sorted({(dy - R) ** 2 + (dx - R) ** 2 for dy in range(K) for dx in range(K)} - {0})
    ds2_col = {v: i for i, v in enumerate(ds2_vals)}

    with TileContext(nc) as tc:
        with (
            tc.tile_pool(name="consts", bufs=1) as cpool,
            tc.tile_pool(name="drampool", bufs=2, space="DRAM") as dpool,
            tc.tile_pool(name="bandpool", bufs=2) as bpool,
            tc.tile_pool(name="workpool", bufs=2) as wpool,
            tc.tile_pool(name="accpool", bufs=2) as apool,
        ):
            bias = cpool.tile([P, len(ds2_vals)], f32)
            for v, i in ds2_col.items():
                nc.gpsimd.memset(bias[:, i : i + 1], -float(v) * INV_2R2)

            with tc.For_i(0, n_img * H, H) as gbase:
                pad = dpool.tile([H + 2 * R, Wp], f32, tag="pad")
                # ---- phase 0: build padded image in DRAM ----
                nc.sync.dma_start(out=pad[R : H + R, R : W + R], in_=x[ds(gbase, H), :])
                with nc.allow_non_contiguous_dma(reason="tiny edge-column pads"):
                    for ccol in range(R):
                        nc.sync.dma_start(out=pad[R : H + R, ccol : ccol + 1], in_=x[ds(gbase, H), 0:1])
                        nc.sync.dma_start(
                            out=pad[R : H + R, W + R + ccol : W + R + ccol + 1],
                            in_=x[ds(gbase, H), W - 1 : W],
                        )
                for rrow in range(R):
                    nc.sync.dma_start(out=pad[rrow : rrow + 1, :], in_=pad[R : R + 1, :])
                    nc.sync.dma_start(
                        out=pad[H + R + rrow : H + R + rrow + 1, :],
                        in_=pad[H + R - 1 : H + R, :],
                    )

                # ---- phase 1: bands ----
                if variant == "bf16A":
                    # process PAIRS of 128-row bands side by side (free dim 2*W)
                    # to amortize per-instruction overhead.
                    assert n_bands % 2 == 0
                    for pb in range(n_bands // 2):
                        r0 = pb * 2 * P
                        Tmap = {}

                        def load_T(dy, r0=r0, Tmap=None):
                            pass

                        def get_T(dy, r0=r0, Tmap=Tmap):
                            if dy not in Tmap:
                                tag = "Tcen" if dy == R else f"T{dy % 3}"
                                t_dy = bpool.tile([P, 2 * Wp], f32, tag=tag)
                                nc.sync.dma_start(out=t_dy[:, 0:Wp], in_=pad[r0 + dy : r0 + dy + P, :])
                                nc.sync.dma_start(out=t_dy[:, Wp : 2 * Wp], in_=pad[r0 + P + dy : r0 + P + dy + P, :])
                                Tmap[dy] = t_dy
                            return Tmap[dy]

                        def seg(tile_, dx):
                            return tile_[:, :].rearrange("p (s c) -> p s c", c=Wp)[:, :, dx : dx + W]

                        cen = seg(get_T(R), R)

                        acc = apool.tile([P, 2 * W], f32, tag="acc")
                        den = apool.tile([P, 2 * W], f32, tag="den")
                        acc3 = acc[:, :].rearrange("p (s c) -> p s c", c=W)
                        nc.vector.memset(acc[:, :], 0.0)
                        nc.gpsimd.memset(den[:, :], 1.0)

                        taps = [(dy, dx) for dy in range(K) for dx in range(K) if not (dy == R and dx == R)]
                        GRP = 8
                        sub_flip = 0

                        def fold_push(stack, tile, eng):
                            lv = 0
                            while stack and stack[-1][0] == lv:
                                _, prev = stack.pop()
                                eng.tensor_tensor(out=prev[:, :], in0=prev[:, :], in1=tile[:, :], op=mybir.AluOpType.add)
                                tile = prev
                                lv += 1
                            stack.append((lv, tile))

                        for g0 in range(0, len(taps), GRP):
                            group = taps[g0 : g0 + GRP]
                            tstack, wstack = [], []
                            for gi, (dy, dx) in enumerate(group):
                                s = seg(get_T(dy), dx)
                                d = wpool.tile([P, 2 * W], bf16, tag=f"d{gi % 2}")
                                q = wpool.tile([P, 2 * W], bf16, tag="q")
                                w = wpool.tile([P, 2 * W], bf16, tag=f"w{gi % 4}")
                                t = wpool.tile([P, 2 * W], bf16, tag=f"t{gi % 4}")
                                d3 = d[:, :].rearrange("p (s c) -> p s c", c=W)
                                sub_eng = nc.gpsimd if (sub_flip % 3 == 2) else nc.vector
                                sub_flip += 1
                                sub_eng.tensor_tensor(out=d3, in0=s, in1=cen, op=mybir.AluOpType.subtract)
                                nc.vector.tensor_tensor(out=q[:, :], in0=d[:, :], in1=d[:, :], op=mybir.AluOpType.mult)
                                ds2 = (dy - R) ** 2 + (dx - R) ** 2
                                nc.scalar.activation(
                                    w[:, :],
                                    q[:, :],
                                    mybir.ActivationFunctionType.Exp,
                                    bias=bias[:, ds2_col[ds2] : ds2_col[ds2] + 1],
                                    scale=-INV_2C2,
                                )
                                nc.vector.tensor_tensor(out=t[:, :], in0=w[:, :], in1=d[:, :], op=mybir.AluOpType.mult)
                                fold_push(tstack, t, nc.vector)
                                fold_push(wstack, w, nc.gpsimd)
                            for stack, accum, eng in ((tstack, acc, nc.vector), (wstack, den, nc.gpsimd)):
                                while len(stack) > 1:
                                    _, b2 = stack.pop()
                                    _, a2 = stack.pop()
                                    eng.tensor_tensor(out=a2[:, :], in0=a2[:, :], in1=b2[:, :], op=mybir.AluOpType.add)
                                    stack.append((99, a2))
                                eng.tensor_tensor(out=accum[:, :], in0=accum[:, :], in1=stack[0][1][:, :], op=mybir.AluOpType.add)

                        rcp = wpool.tile([P, 2 * W], f32, tag="w0")
                        scr = wpool.tile([P, 2 * W], f32, tag="w1")
                        nc.vector.reciprocal_approx_accurate(rcp[:, :], den[:, :], scr[:, :])
                        nc.vector.tensor_tensor(out=acc[:, :], in0=acc[:, :], in1=rcp[:, :], op=mybir.AluOpType.mult)
                        nc.vector.tensor_tensor(out=acc3, in0=acc3, in1=cen, op=mybir.AluOpType.add)
                        nc.sync.dma_start(out=y[ds(gbase + r0, P), :], in_=acc[:, 0:W])
                        nc.sync.dma_start(out=y[ds(gbase + r0 + P, P), :], in_=acc[:, W : 2 * W])
                    continue_images = True  # marker; fp32 path below skipped
                for b in range(n_bands if variant == "fp32" else 0):
                    r0 = b * P
                    T = []
                    for dy in range(K):
                        t_dy = bpool.tile([P, Wp], f32, tag=f"T{dy}")
                        nc.sync.dma_start(out=t_dy[:, :], in_=pad[r0 + dy : r0 + dy + P, :])
                        T.append(t_dy)
                    cen = T[R][:, R : R + W]

                    if variant == "fp32":
                        num = apool.tile([P, W], f32, tag="num")
                        den = apool.tile([P, W], f32, tag="den")
                        nc.scalar.copy(num[:, :], cen)
                        nc.gpsimd.memset(den[:, :], 1.0)

                        for dy in range(K):
                            for dx in range(K):
                                if dy == R and dx == R:
                                    continue
                                s = T[dy][:, dx : dx + W]
                                q = wpool.tile([P, W], f32, tag="q")
                                w = wpool.tile([P, W], f32, tag="w")
                                t = wpool.tile([P, W], f32, tag="t")
                                nc.vector.tensor_tensor(out=q[:, :], in0=s, in1=cen, op=mybir.AluOpType.subtract)
                                nc.vector.tensor_tensor(out=q[:, :], in0=q[:, :], in1=q[:, :], op=mybir.AluOpType.mult)
                                ds2 = (dy - R) ** 2 + (dx - R) ** 2
                                nc.scalar.activation(
                                    w[:, :],
                                    q[:, :],
                                    mybir.ActivationFunctionType.Exp,
                                    bias=bias[:, ds2_col[ds2] : ds2_col[ds2] + 1],
                                    scale=-INV_2C2,
                                )
                                nc.vector.tensor_tensor(out=t[:, :], in0=w[:, :], in1=s, op=mybir.AluOpType.mult)
                                nc.vector.tensor_tensor(out=num[:, :], in0=num[:, :], in1=t[:, :], op=mybir.AluOpType.add)
                                nc.gpsimd.tensor_tensor(out=den[:, :], in0=den[:, :], in1=w[:, :], op=mybir.AluOpType.add)

                        rcp = wpool.tile([P, W], f32, tag="rcp")
                        scr = wpool.tile([P, W], f32, tag="scr")
                        nc.vector.reciprocal_approx_accurate(rcp[:, :], den[:, :], scr[:, :])
                        nc.vector.tensor_tensor(out=num[:, :], in0=num[:, :], in1=rcp[:, :], op=mybir.AluOpType.mult)
                        nc.sync.dma_start(out=y[ds(gbase + r0, P), :], in_=num[:, :])
                    else:  # bf16A
                        acc = apool.tile([P, W], f32, tag="acc")
                        den = apool.tile([P, W], f32, tag="den")
                        nc.vector.memset(acc[:, :], 0.0)
                        nc.gpsimd.memset(den[:, :], 1.0)

                        taps = [(dy, dx) for dy in range(K) for dx in range(K) if not (dy == R and dx == R)]
                        GRP = 8  # taps per bf16 partial-sum tree
                        sub_flip = 0

                        def fold_push(stack, tile, eng):
                            # binary-counter balanced fold: stack holds (level, tile)
                            lv = 0
                            while stack and stack[-1][0] == lv:
                                _, prev = stack.pop()
                                eng.tensor_tensor(out=prev[:, :], in0=prev[:, :], in1=tile[:, :], op=mybir.AluOpType.add)
                                tile = prev
                                lv += 1
                            stack.append((lv, tile))

                        for g0 in range(0, len(taps), GRP):
                            group = taps[g0 : g0 + GRP]
                            tstack, wstack = [], []
                            for gi, (dy, dx) in enumerate(group):
                                s = T[dy][:, dx : dx + W]
                                d = wpool.tile([P, W], bf16, tag=f"d{gi % 2}")
                                q = wpool.tile([P, W], bf16, tag="q")
                                w = wpool.tile([P, W], bf16, tag=f"w{gi % 4}")
                                t = wpool.tile([P, W], bf16, tag=f"t{gi % 4}")
                                # d = s - cen  (fp32 in, bf16 out); 1/3 of subs on gpsimd
                                sub_eng = nc.gpsimd if (sub_flip % 3 == 2) else nc.vector
                                sub_flip += 1
                                sub_eng.tensor_tensor(out=d[:, :], in0=s, in1=cen, op=mybir.AluOpType.subtract)
                                nc.vector.tensor_tensor(out=q[:, :], in0=d[:, :], in1=d[:, :], op=mybir.AluOpType.mult)
                                ds2 = (dy - R) ** 2 + (dx - R) ** 2
                                nc.scalar.activation(
                                    w[:, :],
                                    q[:, :],
                                    mybir.ActivationFunctionType.Exp,
                                    bias=bias[:, ds2_col[ds2] : ds2_col[ds2] + 1],
                                    scale=-INV_2C2,
                                )
                                nc.vector.tensor_tensor(out=t[:, :], in0=w[:, :], in1=d[:, :], op=mybir.AluOpType.mult)
                                fold_push(tstack, t, nc.vector)
                                fold_push(wstack, w, nc.gpsimd)
                            # fold leftovers, then fp32 root add
                            for stack, accum, eng in ((tstack, acc, nc.vector), (wstack, den, nc.gpsimd)):
                                while len(stack) > 1:
                                    _, b2 = stack.pop()
                                    _, a2 = stack.pop()
                                    eng.tensor_tensor(out=a2[:, :], in0=a2[:, :], in1=b2[:, :], op=mybir.AluOpType.add)
                                    stack.append((99, a2))
                                eng.tensor_tensor(out=accum[:, :], in0=accum[:, :], in1=stack[0][1][:, :], op=mybir.AluOpType.add)

                        rcp = wpool.tile([P, W], f32, tag="rcp")
                        scr = wpool.tile([P, W], f32, tag="scr")
                        nc.vector.reciprocal_approx_accurate(rcp[:, :], den[:, :], scr[:, :])
                        nc.vector.tensor_tensor(out=acc[:, :], in0=acc[:, :], in1=rcp[:, :], op=mybir.AluOpType.mult)
                        nc.vector.tensor_tensor(out=acc[:, :], in0=acc[:, :], in1=cen, op=mybir.AluOpType.add)
                        nc.sync.dma_start(out=y[ds(gbase + r0, P), :], in_=acc[:, :])

    nc.finalize()
    return nc


def _get_nc(n_img, H, W, variant="bf16A"):
    key = (n_img, H, W, variant)
    if key not in _NC_CACHE:
        _NC_CACHE[key] = build_nc(n_img, H, W, variant)
    return _NC_CACHE[key]


def run_sharded(flat, n_img_per_core, H, W, trace=False, variant="bf16A"):
    """flat: [N_CORES * n_img_per_core, H, W] fp32. Returns same-shape output
    (and the BassKernelResults when trace)."""
    from concourse.bass_utils import run_bass_kernel_spmd

    nc = _get_nc(n_img_per_core, H, W, variant)
    in_maps = [
        {
            "x": np.ascontiguousarray(
                flat[c * n_img_per_core : (c + 1) * n_img_per_core].reshape(n_img_per_core * H, W)
            )
        }
        for c in range(N_CORES)
    ]
    res = run_bass_kernel_spmd(nc, in_maps, core_ids=list(range(N_CORES)), trace=trace)
    out = np.stack([res.results[c]["y"].reshape(n_img_per_core, H, W) for c in range(N_CORES)])
    return out.reshape(N_CORES * n_img_per_core, H, W), res


def kernel(input_tensor: np.ndarray) -> np.ndarray:
    input_tensor = np.asarray(input_tensor, dtype=np.float32)
    B, C, H, W = input_tensor.shape
    flat = input_tensor.reshape(B * C, H, W)
    assert (B * C) % N_CORES == 0
    out, _ = run_sharded(flat, (B * C) // N_CORES, H, W)
    return out.reshape(B, C, H, W)


# revision 13
# speedup vs baseline: 1.7000x; 1.6556x over previous
"""Bilateral filter (7x7, sigma_color=0.1) Trainium2 Bass kernel.

Full inputs: input_tensor [16, 3, 1024, 1024] fp32 in [0,1].
Sharding: batch-parallel — 48 channel-images split as 6 per core across 8 cores.

Per-core algorithm (one For_i loop over the 6 channel-images):
  1. Build an edge-padded copy [H+6, W+6] in a DRAM-pool tile via DMAs.
  2. For each 128-row band, DMA 7 row-shifted tiles T_dy [128, W+6] from the
     padded image (compute engines cannot shift partitions, DMA can).
  3. Per tap (dy,dx) != center:  d = T_dy[:, dx:dx+W] - center;  q = d*d;
     w = exp(-50*q - ds2/18)  (ACT, spatial weight folded into bias);
     num += w*s (DVE);  den += w (GpSimd).
  4. out = num * approx_reciprocal(den); DMA band to output.
"""

import sys

sys.path.insert(0, "/opt/trn_rl_repo")

import numpy as np

SPATIAL_RADIUS = 3
COLOR_RADIUS = 0.1
INV_2C2 = 1.0 / (2.0 * COLOR_RADIUS**2)  # 50.0
INV_2R2 = 1.0 / (2.0 * float(SPATIAL_RADIUS) ** 2)  # 1/18

N_CORES = 8
_NC_CACHE = {}


def build_nc(n_img, H, W, variant="bf16A"):
    """Build the per-core Bass kernel: n_img channel-images of [H, W].

    variant:
      "fp32"  — all fp32: num/den accumulation of w and w*s.
      "bf16A" — A-formulation out = cen + (sum w*d)/(1 + sum w); d/q/w/t in
                bf16 (DVE 2x mode on the squares/products), A/den in fp32.
    """
    import concourse.bacc as bacc
    import concourse.bass as bass
    import concourse.mybir as mybir
    from concourse.tile import TileContext

    ds = bass.ds
    f32 = mybir.dt.float32
    bf16 = mybir.dt.bfloat16
    K = 2 * SPATIAL_RADIUS + 1  # 7
    R = SPATIAL_RADIUS  # 3
    Wp = W + 2 * R  # padded width
    P = 128  # band height (partitions)
    assert H % P == 0
    n_bands = H // P

    nc = bacc.Bacc(None, target_bir_lowering=False)
    x = nc.declare_dram_parameter("x", [n_img * H, W], f32, isOutput=False)
    y = nc.declare_dram_parameter("y", [n_img * H, W], f32, isOutput=True)

    # distinct spatial-weight classes: ds2 = (dy-R)^2 + (dx-R)^2
    ds2_vals = sorted({(dy - R) ** 2 + (dx - R) ** 2 for dy in range(K) for dx in range(K)} - {0})
    ds2_col = {v: i for i, v in enumerate(ds2_vals)}

    with TileContext(nc) as tc:
        with (
            tc.tile_pool(name="consts", bufs=1) as cpool,
            tc.tile_pool(name="drampool", bufs=2, space="DRAM") as dpool,
            tc.tile_pool(name="bandpool", bufs=2) as bpool,
            tc.tile_pool(name="workpool", bufs=2) as wpool,
            tc.tile_pool(name="accpool", bufs=2) as apool,
        ):
            bias = cpool.tile([P, len(ds2_vals)], f32)
            for v, i in ds2_col.items():
                nc.gpsimd.memset(bias[:, i : i + 1], -float(v) * INV_2R2)

            with tc.For_i(0, n_img * H, H) as gbase:
                pad = dpool.tile([H + 2 * R, Wp], f32, tag="pad")
                # ---- phase 0: build padded image in DRAM ----
                nc.sync.dma_start(out=pad[R : H + R, R : W + R], in_=x[ds(gbase, H), :])
                with nc.allow_non_contiguous_dma(reason="tiny edge-column pads"):
                    for ccol in range(R):
                        nc.sync.dma_start(out=pad[R : H + R, ccol : ccol + 1], in_=x[ds(gbase, H), 0:1])
                        nc.sync.dma_start(
                            out=pad[R : H + R, W + R + ccol : W + R + ccol + 1],
                            in_=x[ds(gbase, H), W - 1 : W],
                        )
                for rrow in range(R):
                    nc.sync.dma_start(out=pad[rrow : rrow + 1, :], in_=pad[R : R + 1, :])
                    nc.sync.dma_start(
                        out=pad[H + R + rrow : H + R + rrow + 1, :],
                        in_=pad[H + R - 1 : H + R, :],
                    )

                # ---- phase 1: bands ----
                if variant == "bf16A":
                    # process PAIRS of 128-row bands side by side (free dim 2*W)
                    # to amortize per-instruction overhead.
                    assert n_bands % 2 == 0
                    for pb in range(n_bands // 2):
                        r0 = pb * 2 * P
                        Tmap = {}

                        def load_T(dy, r0=r0, Tmap=None):
                            pass

                        def get_T(dy, r0=r0, Tmap=Tmap):
                            if dy not in Tmap:
                                tag = "Tcen" if dy == R else f"T{dy % 3}"
                                t_dy = bpool.tile([P, 2 * Wp], f32, tag=tag)
                                nc.sync.dma_start(out=t_dy[:, 0:Wp], in_=pad[r0 + dy : r0 + dy + P, :])
                                nc.sync.dma_start(out=t_dy[:, Wp : 2 * Wp], in_=pad[r0 + P + dy : r0 + P + dy + P, :])
                                Tmap[dy] = t_dy
                            return Tmap[dy]

                        def seg(tile_, dx):
                            return tile_[:, :].rearrange("p (s c) -> p s c", c=Wp)[:, :, dx : dx + W]

                        cen = seg(get_T(R), R)

                        acc = apool.tile([P, 2 * W], f32, tag="acc")
                        den = apool.tile([P, 2 * W], f32, tag="den")
                        acc3 = acc[:, :].rearrange("p (s c) -> p s c", c=W)
                        nc.vector.memset(acc[:, :], 0.0)
                        nc.gpsimd.memset(den[:, :], 1.0)

                        taps = [(dy, dx) for dy in range(K) for dx in range(K) if not (dy == R and dx == R)]
                        GRP = 8
                        sub_flip = 0

                        def fold_push(stack, tile, eng):
                            lv = 0
                            while stack and stack[-1][0] == lv:
                                _, prev = stack.pop()
                                eng.tensor_tensor(out=prev[:, :], in0=prev[:, :], in1=tile[:, :], op=mybir.AluOpType.add)
                                tile = prev
                                lv += 1
                            stack.append((lv, tile))

                        for g0 in range(0, len(taps), GRP):
                            group = taps[g0 : g0 + GRP]
                            tstack, wstack = [], []
                            for gi, (dy, dx) in enumerate(group):
                                s = seg(get_T(dy), dx)
                                d = wpool.tile([P, 2 * W], bf16, tag=f"d{gi % 2}")
                                q = wpool.tile([P, 2 * W], bf16, tag="q")
                                w = wpool.tile([P, 2 * W], bf16, tag=f"w{gi % 4}")
                                t = wpool.tile([P, 2 * W], bf16, tag=f"t{gi % 4}")
                                d3 = d[:, :].rearrange("p (s c) -> p s c", c=W)
                                sub_flip += 1
                                nc.vector.tensor_tensor(out=d3, in0=s, in1=cen, op=mybir.AluOpType.subtract)
                                # square on the (otherwise idle) scalar engine
                                nc.scalar.activation(q[:, :], d[:, :], mybir.ActivationFunctionType.Square)
                                ds2 = (dy - R) ** 2 + (dx - R) ** 2
                                nc.scalar.activation(
                                    w[:, :],
                                    q[:, :],
                                    mybir.ActivationFunctionType.Exp,
                                    bias=bias[:, ds2_col[ds2] : ds2_col[ds2] + 1],
                                    scale=-INV_2C2,
                                )
                                nc.vector.tensor_tensor(out=t[:, :], in0=w[:, :], in1=d[:, :], op=mybir.AluOpType.mult)
                                fold_push(tstack, t, nc.vector)
                                fold_push(wstack, w, nc.vector)
                            for stack, accum, eng in ((tstack, acc, nc.vector), (wstack, den, nc.vector)):
                                while len(stack) > 1:
                                    _, b2 = stack.pop()
                                    _, a2 = stack.pop()
                                    eng.tensor_tensor(out=a2[:, :], in0=a2[:, :], in1=b2[:, :], op=mybir.AluOpType.add)
                                    stack.append((99, a2))
                                eng.tensor_tensor(out=accum[:, :], in0=accum[:, :], in1=stack[0][1][:, :], op=mybir.AluOpType.add)

                        rcp = wpool.tile([P, 2 * W], f32, tag="w0")
                        scr = wpool.tile([P, 2 * W], f32, tag="w1")
                        nc.vector.reciprocal_approx_accurate(rcp[:, :], den[:, :], scr[:, :])
                        nc.vector.tensor_tensor(out=acc[:, :], in0=acc[:, :], in1=rcp[:, :], op=mybir.AluOpType.mult)
                        nc.vector.tensor_tensor(out=acc3, in0=acc3, in1=cen, op=mybir.AluOpType.add)
                        nc.sync.dma_start(out=y[ds(gbase + r0, P), :], in_=acc[:, 0:W])
                        nc.sync.dma_start(out=y[ds(gbase + r0 + P, P), :], in_=acc[:, W : 2 * W])
                    continue_images = True  # marker; fp32 path below skipped
                for b in range(n_bands if variant == "fp32" else 0):
                    r0 = b * P
                    T = []
                    for dy in range(K):
                        t_dy = bpool.tile([P, Wp], f32, tag=f"T{dy}")
                        nc.sync.dma_start(out=t_dy[:, :], in_=pad[r0 + dy : r0 + dy + P, :])
                        T.append(t_dy)
                    cen = T[R][:, R : R + W]

                    if variant == "fp32":
                        num = apool.tile([P, W], f32, tag="num")
                        den = apool.tile([P, W], f32, tag="den")
                        nc.scalar.copy(num[:, :], cen)
                        nc.gpsimd.memset(den[:, :], 1.0)

                        for dy in range(K):
                            for dx in range(K):
                                if dy == R and dx == R:
                                    continue
                                s = T[dy][:, dx : dx + W]
                                q = wpool.tile([P, W], f32, tag="q")
                                w = wpool.tile([P, W], f32, tag="w")
                                t = wpool.tile([P, W], f32, tag="t")
                                nc.vector.tensor_tensor(out=q[:, :], in0=s, in1=cen, op=mybir.AluOpType.subtract)
                                nc.vector.tensor_tensor(out=q[:, :], in0=q[:, :], in1=q[:, :], op=mybir.AluOpType.mult)
                                ds2 = (dy - R) ** 2 + (dx - R) ** 2
                                nc.scalar.activation(
                                    w[:, :],
                                    q[:, :],
                                    mybir.ActivationFunctionType.Exp,
                                    bias=bias[:, ds2_col[ds2] : ds2_col[ds2] + 1],
                                    scale=-INV_2C2,
                                )
                                nc.vector.tensor_tensor(out=t[:, :], in0=w[:, :], in1=s, op=mybir.AluOpType.mult)
                                nc.vector.tensor_tensor(out=num[:, :], in0=num[:, :], in1=t[:, :], op=mybir.AluOpType.add)
                                nc.gpsimd.tensor_tensor(out=den[:, :], in0=den[:, :], in1=w[:, :], op=mybir.AluOpType.add)

                        rcp = wpool.tile([P, W], f32, tag="rcp")
                        scr = wpool.tile([P, W], f32, tag="scr")
                        nc.vector.reciprocal_approx_accurate(rcp[:, :], den[:, :], scr[:, :])
                        nc.vector.tensor_tensor(out=num[:, :], in0=num[:, :], in1=rcp[:, :], op=mybir.AluOpType.mult)
                        nc.sync.dma_start(out=y[ds(gbase + r0, P), :], in_=num[:, :])
                    else:  # bf16A
                        acc = apool.tile([P, W], f32, tag="acc")
                        den = apool.tile([P, W], f32, tag="den")
                        nc.vector.memset(acc[:, :], 0.0)
                        nc.gpsimd.memset(den[:, :], 1.0)

                        taps = [(dy, dx) for dy in range(K) for dx in range(K) if not (dy == R and dx == R)]
                        GRP = 8  # taps per bf16 partial-sum tree
                        sub_flip = 0

                        def fold_push(stack, tile, eng):
                            # binary-counter balanced fold: stack holds (level, tile)
                            lv = 0
                            while stack and stack[-1][0] == lv:
                                _, prev = stack.pop()
                                eng.tensor_tensor(out=prev[:, :], in0=prev[:, :], in1=tile[:, :], op=mybir.AluOpType.add)
                                tile = prev
                                lv += 1
                            stack.append((lv, tile))

                        for g0 in range(0, len(taps), GRP):
                            group = taps[g0 : g0 + GRP]
                            tstack, wstack = [], []
                            for gi, (dy, dx) in enumerate(group):
                                s = T[dy][:, dx : dx + W]
                                d = wpool.tile([P, W], bf16, tag=f"d{gi % 2}")
                                q = wpool.tile([P, W], bf16, tag="q")
                                w = wpool.tile([P, W], bf16, tag=f"w{gi % 4}")
                                t = wpool.tile([P, W], bf16, tag=f"t{gi % 4}")
                                # d = s - cen  (fp32 in, bf16 out); 1/3 of subs on gpsimd
                                sub_eng = nc.gpsimd if (sub_flip % 3 == 2) else nc.vector
                                sub_flip += 1
                                sub_eng.tensor_tensor(out=d[:, :], in0=s, in1=cen, op=mybir.AluOpType.subtract)
                                nc.vector.tensor_tensor(out=q[:, :], in0=d[:, :], in1=d[:, :], op=mybir.AluOpType.mult)
                                ds2 = (dy - R) ** 2 + (dx - R) ** 2
                                nc.scalar.activation(
                                    w[:, :],
                                    q[:, :],
                                    mybir.ActivationFunctionType.Exp,
                                    bias=bias[:, ds2_col[ds2] : ds2_col[ds2] + 1],
                                    scale=-INV_2C2,
                                )
                                nc.vector.tensor_tensor(out=t[:, :], in0=w[:, :], in1=d[:, :], op=mybir.AluOpType.mult)
                                fold_push(tstack, t, nc.vector)
                                fold_push(wstack, w, nc.gpsimd)
                            # fold leftovers, then fp32 root add
                            for stack, accum, eng in ((tstack, acc, nc.vector), (wstack, den, nc.gpsimd)):
                                while len(stack) > 1:
                                    _, b2 = stack.pop()
                                    _, a2 = stack.pop()
                                    eng.tensor_tensor(out=a2[:, :], in0=a2[:, :], in1=b2[:, :], op=mybir.AluOpType.add)
                                    stack.append((99, a2))
                                eng.tensor_tensor(out=accum[:, :], in0=accum[:, :], in1=stack[0][1][:, :], op=mybir.AluOpType.add)

                        rcp = wpool.tile([P, W], f32, tag="rcp")
                        scr = wpool.tile([P, W], f32, tag="scr")
                        nc.vector.reciprocal_approx_accurate(rcp[:, :], den[:, :], scr[:, :])
                        nc.vector.tensor_tensor(out=acc[:, :], in0=acc[:, :], in1=rcp[:, :], op=mybir.AluOpType.mult)
                        nc.vector.tensor_tensor(out=acc[:, :], in0=acc[:, :], in1=cen, op=mybir.AluOpType.add)
                        nc.sync.dma_start(out=y[ds(gbase + r0, P), :], in_=acc[:, :])

    nc.finalize()
    return nc


def _get_nc(n_img, H, W, variant="bf16A"):
    key = (n_img, H, W, variant)
    if key not in _NC_CACHE:
        _NC_CACHE[key] = build_nc(n_img, H, W, variant)
    return _NC_CACHE[key]


def run_sharded(flat, n_img_per_core, H, W, trace=False, variant="bf16A"):
    """flat: [N_CORES * n_img_per_core, H, W] fp32. Returns same-shape output
    (and the BassKernelResults when trace)."""
    from concourse.bass_utils import run_bass_kernel_spmd

    nc = _get_nc(n_img_per_core, H, W, variant)
    in_maps = [
        {
            "x": np.ascontiguousarray(
                flat[c * n_img_per_core : (c + 1) * n_img_per_core].reshape(n_img_per_core * H, W)
            )
        }
        for c in range(N_CORES)
    ]
    res = run_bass_kernel_spmd(nc, in_maps, core_ids=list(range(N_CORES)), trace=trace)
    out = np.stack([res.results[c]["y"].reshape(n_img_per_core, H, W) for c in range(N_CORES)])
    return out.reshape(N_CORES * n_img_per_core, H, W), res


def kernel(input_tensor: np.ndarray) -> np.ndarray:
    input_tensor = np.asarray(input_tensor, dtype=np.float32)
    B, C, H, W = input_tensor.shape
    flat = input_tensor.reshape(B * C, H, W)
    assert (B * C) % N_CORES == 0
    out, _ = run_sharded(flat, (B * C) // N_CORES, H, W)
    return out.reshape(B, C, H, W)
